# revision 3
# baseline (speedup 1.0000x reference)
"""Multi-head attention (B=4, N=2048, D=1024, H=16) on 8 Trainium2 cores.

Sharding: core = (batch b, head-group hg) -> 4 batches x 2 groups of 8 heads.
Each core computes, for its batch and its 8 heads, with zero on-device
transposes of the inputs (the host feeds x pre-transposed):
  - K^T, V, Q^T projections (K^T/Q^T in [feature, token] layout so they feed
    the scores matmuls directly; V in natural [token, feature] layout with a
    ones-column appended per 65-wide head slot),
  - scores S^T[j, i] via row-packed K=64 matmul pairs,
  - exp on the scalar engine (no max-subtraction needed: scores are
    ~N(0, 0.17), and softmax is shift-invariant),
  - PV in flipped orientation: O[i, d] with M=128 output partitions (cost on
    the PE is proportional to the moving free size only, so M=128 halves PE
    time vs the M=65 O^T form).  lhsT = P~ slice [j, i-chunk], rhs = V with
    the ones column, so column 64 of the accumulator is the softmax
    denominator for free,
  - deferred normalization: O * broadcast(1/denom) on the DVE via
    tensor_scalar (the denominator is a per-partition scalar in this
    orientation - no gpsimd partition broadcast needed),
  - PE transposes of the normalized [i, d-pair] tiles back to O^T feeding
    the output projection (8 transposes packed into one PSUM s-slot, one
    DVE copy per unit),
  - a partial output projection against its 512 rows of W_proj.
Host sums the two per-batch partials and adds b_proj.

All matmuls run in fp32r except PV/transpose/proj-lhsT which run in bf16.
The emission order software-pipelines the kernel: a merged K^T+V pass over
one x^T stream -> Q^T for token halves n0/n1 -> attention units, with Q
n2/n3 and the block-0 output projection emitted as boundary fillers.
W_proj loads into the weight-stream pool's bytes after the last Q filler.
"""

import sys

if "/opt/trn_rl_repo" not in sys.path:
    sys.path.insert(0, "/opt/trn_rl_repo")

from contextlib import ExitStack

import numpy as np

B, N, D, H = 4, 2048, 1024, 16
HG = 2                 # head groups (tensor parallel)
NCORES = B * HG        # 8
DH = D // HG           # 512 features per group = 8 heads * 64
P = 128
KC = D // P            # 8 contraction chunks over d_model
MC = 2 * DH // P       # 8 feature chunks of [Q|K]
NT = N // 512          # 4 token 512-chunks
TJ = N // P            # 16 token 128-chunks (the attention j axis)
IB = 1024              # i-block (exp free-dim)
NI = N // IB           # 2
IQ = IB // 512         # 2 matmul free-dim quarters per i-block
IC = IB // P           # 8 i-chunks of 128 per i-block
CP = 4                 # head pairs per core
SCALE = (D // H) ** -0.5

_cached = {}


def _build():
    import concourse.mybir as mybir
    import concourse.tile as tile
    from concourse import bacc
    from concourse.masks import make_identity

    f32 = mybir.dt.float32
    f32r = mybir.dt.float32r
    bf16 = mybir.dt.bfloat16
    AF = mybir.ActivationFunctionType

    nc = bacc.Bacc("TRN2", target_bir_lowering=False, debug=False,
                   enable_asserts=False)

    xt = nc.dram_tensor("xt", (D, N), f32r, kind="ExternalInput").ap()
    wqk = nc.dram_tensor("wqk", (D, 2 * DH), f32r, kind="ExternalInput").ap()
    wv = nc.dram_tensor("wv", (D, DH), f32r, kind="ExternalInput").ap()
    wp = nc.dram_tensor("wp", (DH, D), f32r, kind="ExternalInput").ap()
    bqk = nc.dram_tensor("bqk", (1, 2 * DH), f32, kind="ExternalInput").ap()
    bv = nc.dram_tensor("bv", (1, DH), f32r, kind="ExternalInput").ap()
    y = nc.dram_tensor("y", (N, D), f32, kind="ExternalOutput").ap()

    with tile.TileContext(nc) as tc, ExitStack() as ctx:
        const = ctx.enter_context(tc.tile_pool(name="const", bufs=1))
        persist = ctx.enter_context(tc.tile_pool(name="persist", bufs=1))
        ppool = ctx.enter_context(tc.tile_pool(name="pp", bufs=3))
        otpool = ctx.enter_context(tc.tile_pool(name="ot", bufs=2))
        dpool = ctx.enter_context(tc.tile_pool(name="dv", bufs=2))
        onpool = ctx.enter_context(tc.tile_pool(name="on", bufs=3))
        ypool = ctx.enter_context(tc.tile_pool(name="yb", bufs=3))
        # streaming pools shared by the three projection passes
        xpool = ctx.enter_context(tc.tile_pool(name="xs", bufs=2))
        ws_stack = ExitStack()
        wpool = ws_stack.enter_context(tc.tile_pool(name="ws", bufs=1))
        # single PSUM pool: tag "s" = 2x [128, IB] (scores / qkv / proj /
        # transpose-out), oa/ob = per-head O[i, d] accumulators
        # ([128, 8, 128] = 2 banks each) -> exactly 8 banks
        psp = ctx.enter_context(tc.tile_pool(name="psp", bufs=2, space="PSUM"))

        ones_f32 = const.tile([1, P], f32)
        nc.vector.memset(ones_f32[:], 1.0)
        ones_row = const.tile([1, P], f32r)
        nc.vector.tensor_copy(ones_row[:], ones_f32[:])
        bqk_sb = const.tile([P, 1, MC], f32)
        nc.sync.dma_start(bqk_sb[:], bqk.rearrange("a (mo p) -> p a mo", p=P))
        bv_sb = const.tile([1, DH], f32r)
        nc.sync.dma_start(bv_sb[:], bv)
        ident = const.tile([P, P], bf16)
        make_identity(nc, ident[:])
        # preload the exp table set during the projection phase
        dummy = const.tile([1, 16], f32)
        nc.scalar.activation(dummy[:], ones_f32[0:1, 0:16], AF.Exp)

        qt = persist.tile([P, MC // 2, N], f32r)      # Q^T  [128, 4, 2048]
        kt = persist.tile([P, MC // 2, N], f32r)      # K^T  [128, 4, 2048]
        # V with a ones column per head (65-wide head slots): the flipped
        # PV matmul then yields O[i, 0:64] plus the softmax denominator in
        # column 64.
        vsb = persist.tile([P, TJ, H // HG, 65], bf16)
        nc.vector.memset(vsb[:, :, :, 64:65], 1.0)

        xt_r = xt.rearrange("(ko p) t -> p ko t", p=P)
        wqk_r = wqk.rearrange("(ko p) m -> p ko m", p=P)
        wv_r = wv.rearrange("(ko p) m -> p ko m", p=P)

        # ---- Pass 1: K^T and V, merged over one x^T stream (wk and wv
        # are co-resident; reversed n-order so the pass ends holding the
        # n1/n0 tiles the Q pass needs) ----
        # interleave the wk / x^T chunk DMAs per k so the first K matmul's
        # k-accumulation can stream as transfers land
        wk_sb = wpool.tile([P, KC, DH], f32r, tag="w", bufs=2, name="wk_sb")
        xk_first = xpool.tile([P, KC, 512], f32r, tag="xt", name="xt_k")
        for k in range(KC):
            nc.sync.dma_start(wk_sb[:, k, :], wqk_r[:, k, DH:2 * DH])
            nc.sync.dma_start(xk_first[:, k, :],
                              xt_r[:, k, (NT - 1) * 512:NT * 512])
        wv_sb = wpool.tile([P, KC, DH], f32r, tag="w", bufs=2, name="wv_sb")
        for k in range(KC):
            nc.sync.dma_start(wv_sb[:, k, :], wv_r[:, k, :])
        xv_tiles = {}
        for n in range(NT - 1, -1, -1):
            if n == NT - 1:
                xt_t = xk_first
            else:
                xt_t = xpool.tile([P, KC, 512], f32r, tag="xt", name="xt_k")
                for k in range(KC):
                    nc.sync.dma_start(xt_t[:, k, :],
                                      xt_r[:, k, n * 512:(n + 1) * 512])
            xv_tiles[n] = xt_t
            for m in range(MC // 2):
                pt = psp.tile([P, IB], f32, tag="s", name="pt")
                for k in range(KC):
                    nc.tensor.matmul(pt[:, 0:512],
                                     wk_sb[:, k, m * P:(m + 1) * P],
                                     xt_t[:, k, :], start=(k == 0),
                                     stop=(k == KC - 1))
                nc.vector.tensor_scalar_add(
                    kt[:, m, n * 512:(n + 1) * 512], pt[:, 0:512],
                    bqk_sb[:, 0, (MC // 2) + m:(MC // 2) + m + 1])
            for tt in range(4):
                t = n * 4 + tt
                pv = psp.tile([P, IB], f32, tag="s", name="pv")
                for k in range(KC):
                    nc.tensor.matmul(pv[:, 0:DH],
                                     xt_t[:, k, tt * P:(tt + 1) * P],
                                     wv_sb[:, k, :], start=(k == 0),
                                     stop=False)
                nc.tensor.matmul(pv[:, 0:DH], ones_row[:], bv_sb[:],
                                 start=False, stop=True)
                nc.vector.tensor_copy(
                    vsb[:, t, :, 0:64],
                    pv[:, 0:DH].rearrange("p (h d) -> p h d", d=64))

        # ---- Pass 3: Q^T, n-outer. Token chunks n0+n1 (= i-block 0) are
        # emitted up front so attention can start; n2+n3 are emitted after
        # the first attention unit and act as PE filler ----
        wq_sb = wpool.tile([P, KC, DH], f32r, tag="w", bufs=2, name="wq_sb")
        for k in range(KC):
            nc.sync.dma_start(wq_sb[:, k, :], wqk_r[:, k, 0:DH])

        def emit_q_n(n):
            if n <= 1:
                xt_t = xv_tiles[n]      # still resident from the V pass
            else:
                xt_t = xpool.tile([P, KC, 512], f32r, tag="xt", name="xt_q")
                for k in range(KC):
                    nc.sync.dma_start(xt_t[:, k, :],
                                      xt_r[:, k, n * 512:(n + 1) * 512])
            for m in range(MC // 2):
                pt = psp.tile([P, IB], f32, tag="s", name="pt")
                for k in range(KC):
                    nc.tensor.matmul(pt[:, 0:512],
                                     wq_sb[:, k, m * P:(m + 1) * P],
                                     xt_t[:, k, :], start=(k == 0),
                                     stop=(k == KC - 1))
                nc.vector.tensor_scalar_add(
                    qt[:, m, n * 512:(n + 1) * 512], pt[:, 0:512],
                    bqk_sb[:, 0, m:m + 1])

        emit_q_n(0)
        emit_q_n(1)

        # token chunks n2/n3 of Q^T are computed as boundary fillers during
        # the first attention block; prefetch their x^T tiles now
        from collections import deque
        fillers = deque()
        xq_late = {}
        for n in (2, 3):
            xt_t = xpool.tile([P, KC, 512], f32r, tag="xt", name="xt_qf")
            for k in range(KC):
                nc.sync.dma_start(xt_t[:, k, :],
                                  xt_r[:, k, n * 512:(n + 1) * 512])
            xq_late[n] = xt_t

        def make_q_filler(n, m):
            def f():
                pt = psp.tile([P, IB], f32, tag="s", name="pt_f")
                for k in range(KC):
                    nc.tensor.matmul(pt[:, 0:512],
                                     wq_sb[:, k, m * P:(m + 1) * P],
                                     xq_late[n][:, k, :], start=(k == 0),
                                     stop=(k == KC - 1))
                nc.vector.tensor_scalar_add(
                    qt[:, m, n * 512:(n + 1) * 512], pt[:, 0:512],
                    bqk_sb[:, 0, m:m + 1])
            return f

        for n in (2, 3):
            for m in range(MC // 2):
                fillers.append(make_q_filler(n, m))

        # ---------------- Attention ----------------
        with ExitStack() as c3:
            wp_sb = None    # allocated after the weight-stream pool closes

            def make_proj_filler(i, ot_blk, t, o):
                def f():
                    yp_full = psp.tile([P, IB], f32, tag="s", name="yp")
                    yp = yp_full[:, 0:512]
                    for cc in range(CP):
                        nc.tensor.matmul(
                            yp[:], ot_blk[:, cc, t * P:(t + 1) * P],
                            wp_sb[:, cc, o * 512:(o + 1) * 512],
                            start=(cc == 0), stop=(cc == CP - 1))
                    ysb = ypool.tile([P, 512], f32, tag="y")
                    nc.vector.tensor_copy(ysb[:], yp[:])
                    r0 = i * IB + t * P
                    nc.sync.dma_start(
                        y[r0:r0 + P, o * 512:(o + 1) * 512], ysb[:])
                return f

            def emit_proj(i, ot_blk):
                for t in range(IB // P):
                    for o in range(D // 512):
                        fillers.append(make_proj_filler(i, ot_blk, t, o))

            for i in range(NI):
                ot_i = otpool.tile([P, CP, IB], bf16, tag="ot")
                for c in range(CP):
                    def emit_scores_exp(i, c, j):
                        s_a = psp.tile([P, IB], f32, tag="s", name="s_a")
                        s_b = psp.tile([P, IB], f32, tag="s", name="s_b")
                        ksl = slice(j * P, (j + 1) * P)
                        for iq in range(IQ):
                            isl = slice(i * IB + iq * 512, i * IB + (iq + 1) * 512)
                            osl = slice(iq * 512, (iq + 1) * 512)
                            # row-packed score matmuls: head A rows 0:64,
                            # head B rows 64:128 of qk feature chunk c
                            nc.tensor.matmul(s_a[:, osl], kt[0:64, c, ksl],
                                             qt[0:64, c, isl], start=True, stop=True)
                            nc.tensor.matmul(s_b[:, osl], kt[64:128, c, ksl],
                                             qt[64:128, c, isl], start=True, stop=True)
                        p_a = ppool.tile([P, IB], bf16, tag="p", name="p_a")
                        nc.scalar.activation(p_a[:], s_a[:], AF.Exp, scale=SCALE)
                        p_b = ppool.tile([P, IB], bf16, tag="p", name="p_b")
                        nc.scalar.activation(p_b[:], s_b[:], AF.Exp, scale=SCALE)
                        return p_a, p_b

                    def emit_pv(oa, ob, p_a, p_b, j):
                        st = (j == 0)
                        sp = (j == TJ - 1)
                        # flipped PV: out O[i-chunk, d], lhsT = P~ slice,
                        # rhs = V||ones; column 64 accumulates the softmax
                        # denominator
                        for ic in range(IC):
                            nc.tensor.matmul(oa[:, ic, 0:65],
                                             p_a[:, ic * P:(ic + 1) * P],
                                             vsb[:, j, 2 * c, :],
                                             start=st, stop=sp)
                        for ic in range(IC):
                            nc.tensor.matmul(ob[:, ic, 0:65],
                                             p_b[:, ic * P:(ic + 1) * P],
                                             vsb[:, j, 2 * c + 1, :],
                                             start=st, stop=sp)

                    # 1-j head start + scores(j+1) emitted before PV(j):
                    # decouples the exp stream from the oa/ob slot-release
                    # chain of the previous unit
                    head0 = emit_scores_exp(i, c, 0)
                    for _ in range(2):
                        if fillers:
                            fillers.popleft()()
                    if i == 0 and c == CP - 1:
                        # all Q fillers have been emitted; release the
                        # weight-stream pool and load W_proj into its bytes
                        ws_stack.close()
                        p2 = c3.enter_context(tc.tile_pool(name="p2", bufs=1))
                        wp_sb = p2.tile([P, DH // P, D], f32r)
                        nc.sync.dma_start(
                            wp_sb[:], wp.rearrange("(c p) o -> p c o", p=P))
                    oa = psp.tile([P, IC, P], f32, tag="oa", bufs=1, name="oa")
                    ob = psp.tile([P, IC, P], f32, tag="ob", bufs=1, name="ob")
                    p_prev = head0
                    for j in range(1, TJ):
                        p_cur = emit_scores_exp(i, c, j)
                        emit_pv(oa, ob, p_prev[0], p_prev[1], j - 1)
                        p_prev = p_cur
                    emit_pv(oa, ob, p_prev[0], p_prev[1], TJ - 1)
                    # deferred softmax normalization: O[i, d] * (1/denom),
                    # denom is a per-partition scalar (column 64)
                    ra = dpool.tile([P, IC], f32, tag="ra")
                    nc.vector.reciprocal(ra[:], oa[:, :, 64])
                    rb = dpool.tile([P, IC], f32, tag="rb")
                    nc.vector.reciprocal(rb[:], ob[:, :, 64])
                    # normalize + transpose each [128 i, 128 d-pair] tile
                    # back to O^T; all 8 transposes pack into one s-slot
                    tp = psp.tile([P, IB], f32, tag="s", name="tp")
                    for ic in range(IC):
                        on = onpool.tile([P, P], bf16, tag="on")
                        nc.vector.tensor_scalar_mul(
                            on[:, 0:64], oa[:, ic, 0:64], ra[:, ic:ic + 1])
                        nc.vector.tensor_scalar_mul(
                            on[:, 64:128], ob[:, ic, 0:64], rb[:, ic:ic + 1])
                        nc.tensor.transpose(
                            tp[:, 64 * ic:64 * (ic + 1)].bitcast(bf16),
                            on[:], ident[:])
                    nc.vector.tensor_copy(ot_i[:, c, :],
                                          tp[:, 0:512].bitcast(bf16))
                emit_proj(i, ot_i)
            # drain remaining fillers (tail projection work)
            while fillers:
                fillers.popleft()()

    nc.compile()
    return nc


def _get_nc():
    if "nc" not in _cached:
        _cached["nc"] = _build()
    return _cached["nc"]


def kernel(x, W_qkv, b_qkv, W_proj, b_proj):
    from concourse.bass_utils import run_bass_kernel_spmd

    x = np.asarray(x, dtype=np.float32)
    W_qkv = np.asarray(W_qkv, dtype=np.float32)
    b_qkv = np.asarray(b_qkv, dtype=np.float32)
    W_proj = np.asarray(W_proj, dtype=np.float32)
    b_proj = np.asarray(b_proj, dtype=np.float32)

    in_maps = []
    for core in range(NCORES):
        b, hg = divmod(core, HG)
        hs = slice(DH * hg, DH * (hg + 1))
        in_maps.append({
            "xt": np.ascontiguousarray(x[b].T),
            "wqk": np.ascontiguousarray(
                np.concatenate([W_qkv[:, hs],
                                W_qkv[:, D + DH * hg:D + DH * (hg + 1)]], axis=1)),
            "wv": np.ascontiguousarray(W_qkv[:, 2 * D + DH * hg:2 * D + DH * (hg + 1)]),
            "wp": np.ascontiguousarray(W_proj[DH * hg:DH * (hg + 1), :]),
            "bqk": np.concatenate([b_qkv[hs],
                                   b_qkv[D + DH * hg:D + DH * (hg + 1)]])[None, :],
            "bv": b_qkv[2 * D + DH * hg:2 * D + DH * (hg + 1)][None, :],
        })

    nc = _get_nc()
    res = run_bass_kernel_spmd(nc, in_maps, core_ids=list(range(NCORES)))
    out = np.empty((B, N, D), dtype=np.float32)
    for b in range(B):
        out[b] = res.results[2 * b]["y"] + res.results[2 * b + 1]["y"] + b_proj
    return out


# revision 9
# speedup vs baseline: 1.0000x; 1.0000x over previous
"""Multi-head attention (B=4, N=2048, D=1024, H=16) on 8 Trainium2 cores.

Sharding: core = (batch b, head-group hg) -> 4 batches x 2 groups of 8 heads.
Each core computes, for its batch and its 8 heads, with zero on-device
transposes of the inputs (the host feeds x pre-transposed):
  - K^T, V, Q^T projections (K^T/Q^T in [feature, token] layout so they feed
    the scores matmuls directly; V in natural [token, feature] layout with a
    ones-column appended per 65-wide head slot),
  - scores S^T[j, i] via row-packed K=64 matmul pairs,
  - exp on the scalar engine (no max-subtraction needed: scores are
    ~N(0, 0.17), and softmax is shift-invariant),
  - PV in flipped orientation: O[i, d] with M=128 output partitions (cost on
    the PE is proportional to the moving free size only, so M=128 halves PE
    time vs the M=65 O^T form).  lhsT = P~ slice [j, i-chunk], rhs = V with
    the ones column, so column 64 of the accumulator is the softmax
    denominator for free,
  - deferred normalization: O * broadcast(1/denom) on the DVE via
    tensor_scalar (the denominator is a per-partition scalar in this
    orientation - no gpsimd partition broadcast needed),
  - PE transposes of the normalized [i, d-pair] tiles back to O^T feeding
    the output projection (8 transposes packed into one PSUM s-slot, one
    DVE copy per unit),
  - a partial output projection against its 512 rows of W_proj.
Host sums the two per-batch partials and adds b_proj.

All matmuls run in fp32r except PV/transpose/proj-lhsT which run in bf16.
The emission order software-pipelines the kernel: a merged K^T+V pass over
one x^T stream -> Q^T for token halves n0/n1 -> attention units, with Q
n2/n3 and the block-0 output projection emitted as boundary fillers.
W_proj loads into the weight-stream pool's bytes after the last Q filler.
"""

import sys

if "/opt/trn_rl_repo" not in sys.path:
    sys.path.insert(0, "/opt/trn_rl_repo")

from contextlib import ExitStack

import ml_dtypes
import numpy as np

B, N, D, H = 4, 2048, 1024, 16
HG = 2                 # head groups (tensor parallel)
NCORES = B * HG        # 8
DH = D // HG           # 512 features per group = 8 heads * 64
P = 128
KC = D // P            # 8 contraction chunks over d_model
MC = 2 * DH // P       # 8 feature chunks of [Q|K]
NT = N // 512          # 4 token 512-chunks
TJ = N // P            # 16 token 128-chunks (the attention j axis)
IB = 1024              # i-block (exp free-dim)
NI = N // IB           # 2
IQ = IB // 512         # 2 matmul free-dim quarters per i-block
IC = IB // P           # 8 i-chunks of 128 per i-block
CP = 4                 # head pairs per core
SCALE = (D // H) ** -0.5

_cached = {}


def _build():
    import concourse.mybir as mybir
    import concourse.tile as tile
    from concourse import bacc
    from concourse.masks import make_identity

    f32 = mybir.dt.float32
    f32r = mybir.dt.float32r
    bf16 = mybir.dt.bfloat16
    AF = mybir.ActivationFunctionType

    nc = bacc.Bacc("TRN2", target_bir_lowering=False, debug=False,
                   enable_asserts=False)

    xt = nc.dram_tensor("xt", (D, N), f32r, kind="ExternalInput").ap()
    wqk = nc.dram_tensor("wqk", (D, 2 * DH), f32r, kind="ExternalInput").ap()
    wv = nc.dram_tensor("wv", (D, DH), f32r, kind="ExternalInput").ap()
    wp = nc.dram_tensor("wp", (DH, D), bf16, kind="ExternalInput").ap()
    bqk = nc.dram_tensor("bqk", (1, 2 * DH), f32, kind="ExternalInput").ap()
    bv = nc.dram_tensor("bv", (1, DH), f32r, kind="ExternalInput").ap()
    y = nc.dram_tensor("y", (N, D), f32, kind="ExternalOutput").ap()

    with tile.TileContext(nc) as tc, ExitStack() as ctx:
        const = ctx.enter_context(tc.tile_pool(name="const", bufs=1))
        persist = ctx.enter_context(tc.tile_pool(name="persist", bufs=1))
        ppool = ctx.enter_context(tc.tile_pool(name="pp", bufs=3))
        otpool = ctx.enter_context(tc.tile_pool(name="ot", bufs=2))
        dpool = ctx.enter_context(tc.tile_pool(name="dv", bufs=2))
        onpool = ctx.enter_context(tc.tile_pool(name="on", bufs=3))
        ypool = ctx.enter_context(tc.tile_pool(name="yb", bufs=3))
        # streaming pools shared by the three projection passes
        xpool = ctx.enter_context(tc.tile_pool(name="xs", bufs=2))
        ws_stack = ExitStack()
        wpool = ws_stack.enter_context(tc.tile_pool(name="ws", bufs=1))
        # single PSUM pool: tag "s" = 2x [128, IB] (scores / qkv / proj /
        # transpose-out), oa/ob = per-head O[i, d] accumulators
        # ([128, 8, 128] = 2 banks each) -> exactly 8 banks
        psp = ctx.enter_context(tc.tile_pool(name="psp", bufs=2, space="PSUM"))

        ones_f32 = const.tile([1, P], f32)
        nc.vector.memset(ones_f32[:], 1.0)
        ones_row = const.tile([1, P], f32r)
        nc.vector.tensor_copy(ones_row[:], ones_f32[:])
        bqk_sb = const.tile([P, 1, MC], f32)
        nc.sync.dma_start(bqk_sb[:], bqk.rearrange("a (mo p) -> p a mo", p=P))
        bv_sb = const.tile([1, DH], f32r)
        nc.sync.dma_start(bv_sb[:], bv)
        ident = const.tile([P, P], bf16)
        make_identity(nc, ident[:])
        # preload the exp table set during the projection phase
        dummy = const.tile([1, 16], f32)
        nc.scalar.activation(dummy[:], ones_f32[0:1, 0:16], AF.Exp)

        qt = persist.tile([P, MC // 2, N], f32r)      # Q^T  [128, 4, 2048]
        kt = persist.tile([P, MC // 2, N], f32r)      # K^T  [128, 4, 2048]
        # V with a ones column per head (65-wide head slots): the flipped
        # PV matmul then yields O[i, 0:64] plus the softmax denominator in
        # column 64.
        vsb = persist.tile([P, TJ, H // HG, 65], bf16)
        nc.vector.memset(vsb[:, :, :, 64:65], 1.0)

        xt_r = xt.rearrange("(ko p) t -> p ko t", p=P)
        wqk_r = wqk.rearrange("(ko p) m -> p ko m", p=P)
        wv_r = wv.rearrange("(ko p) m -> p ko m", p=P)

        # ---- Pass 1: K^T and V, merged over one x^T stream (wk and wv
        # are co-resident; reversed n-order so the pass ends holding the
        # n1/n0 tiles the Q pass needs) ----
        # interleave the wk / x^T chunk DMAs per k so the first K matmul's
        # k-accumulation can stream as transfers land
        wk_sb = wpool.tile([P, KC, DH], f32r, tag="w", bufs=2, name="wk_sb")
        xk_first = xpool.tile([P, KC, 512], f32r, tag="xt", name="xt_k")
        for k in range(KC):
            nc.sync.dma_start(wk_sb[:, k, :], wqk_r[:, k, DH:2 * DH])
            nc.sync.dma_start(xk_first[:, k, :],
                              xt_r[:, k, (NT - 1) * 512:NT * 512])
        wv_sb = wpool.tile([P, KC, DH], f32r, tag="w", bufs=2, name="wv_sb")
        for k in range(KC):
            nc.sync.dma_start(wv_sb[:, k, :], wv_r[:, k, :])
        xv_tiles = {}
        for n in range(NT - 1, -1, -1):
            if n == NT - 1:
                xt_t = xk_first
            else:
                xt_t = xpool.tile([P, KC, 512], f32r, tag="xt", name="xt_k")
                for k in range(KC):
                    nc.sync.dma_start(xt_t[:, k, :],
                                      xt_r[:, k, n * 512:(n + 1) * 512])
            xv_tiles[n] = xt_t
            for m in range(MC // 2):
                pt = psp.tile([P, IB], f32, tag="s", name="pt")
                for k in range(KC):
                    nc.tensor.matmul(pt[:, 0:512],
                                     wk_sb[:, k, m * P:(m + 1) * P],
                                     xt_t[:, k, :], start=(k == 0),
                                     stop=(k == KC - 1))
                nc.vector.tensor_scalar_add(
                    kt[:, m, n * 512:(n + 1) * 512], pt[:, 0:512],
                    bqk_sb[:, 0, (MC // 2) + m:(MC // 2) + m + 1])
            for tt in range(4):
                t = n * 4 + tt
                pv = psp.tile([P, IB], f32, tag="s", name="pv")
                for k in range(KC):
                    nc.tensor.matmul(pv[:, 0:DH],
                                     xt_t[:, k, tt * P:(tt + 1) * P],
                                     wv_sb[:, k, :], start=(k == 0),
                                     stop=False)
                nc.tensor.matmul(pv[:, 0:DH], ones_row[:], bv_sb[:],
                                 start=False, stop=True)
                nc.vector.tensor_copy(
                    vsb[:, t, :, 0:64],
                    pv[:, 0:DH].rearrange("p (h d) -> p h d", d=64))

        # ---- Pass 3: Q^T, n-outer. Token chunks n0+n1 (= i-block 0) are
        # emitted up front so attention can start; n2+n3 are emitted after
        # the first attention unit and act as PE filler ----
        wq_sb = wpool.tile([P, KC, DH], f32r, tag="w", bufs=2, name="wq_sb")
        for k in range(KC):
            nc.sync.dma_start(wq_sb[:, k, :], wqk_r[:, k, 0:DH])

        def emit_q_n(n):
            if n <= 1:
                xt_t = xv_tiles[n]      # still resident from the V pass
            else:
                xt_t = xpool.tile([P, KC, 512], f32r, tag="xt", name="xt_q")
                for k in range(KC):
                    nc.sync.dma_start(xt_t[:, k, :],
                                      xt_r[:, k, n * 512:(n + 1) * 512])
            for m in range(MC // 2):
                pt = psp.tile([P, IB], f32, tag="s", name="pt")
                for k in range(KC):
                    nc.tensor.matmul(pt[:, 0:512],
                                     wq_sb[:, k, m * P:(m + 1) * P],
                                     xt_t[:, k, :], start=(k == 0),
                                     stop=(k == KC - 1))
                nc.vector.tensor_scalar_add(
                    qt[:, m, n * 512:(n + 1) * 512], pt[:, 0:512],
                    bqk_sb[:, 0, m:m + 1])

        emit_q_n(0)
        emit_q_n(1)

        # token chunks n2/n3 of Q^T are computed as boundary fillers during
        # the first attention block; prefetch their x^T tiles now
        from collections import deque
        fillers = deque()
        xq_late = {}
        for n in (2, 3):
            xt_t = xpool.tile([P, KC, 512], f32r, tag="xt", name="xt_qf")
            for k in range(KC):
                nc.sync.dma_start(xt_t[:, k, :],
                                  xt_r[:, k, n * 512:(n + 1) * 512])
            xq_late[n] = xt_t

        def make_q_filler(n, m):
            def f():
                pt = psp.tile([P, IB], f32, tag="s", name="pt_f")
                for k in range(KC):
                    nc.tensor.matmul(pt[:, 0:512],
                                     wq_sb[:, k, m * P:(m + 1) * P],
                                     xq_late[n][:, k, :], start=(k == 0),
                                     stop=(k == KC - 1))
                nc.vector.tensor_scalar_add(
                    qt[:, m, n * 512:(n + 1) * 512], pt[:, 0:512],
                    bqk_sb[:, 0, m:m + 1])
            return f

        for n in (2, 3):
            for m in range(MC // 2):
                fillers.append(make_q_filler(n, m))

        # ---------------- Attention ----------------
        with ExitStack() as c3:
            wp_sb = None    # allocated after the weight-stream pool closes

            def make_proj_filler(i, ot_blk, t, o):
                def f():
                    yp_full = psp.tile([P, IB], f32, tag="s", name="yp")
                    yp = yp_full[:, 0:512]
                    for cc in range(CP):
                        nc.tensor.matmul(
                            yp[:], ot_blk[:, cc, t * P:(t + 1) * P],
                            wp_sb[:, cc, o * 512:(o + 1) * 512],
                            start=(cc == 0), stop=(cc == CP - 1))
                    ysb = ypool.tile([P, 512], f32, tag="y")
                    nc.vector.tensor_copy(ysb[:], yp[:])
                    r0 = i * IB + t * P
                    nc.sync.dma_start(
                        y[r0:r0 + P, o * 512:(o + 1) * 512], ysb[:])
                return f

            def emit_proj(i, ot_blk):
                for t in range(IB // P):
                    for o in range(D // 512):
                        fillers.append(make_proj_filler(i, ot_blk, t, o))

            for i in range(NI):
                ot_i = otpool.tile([P, CP, IB], bf16, tag="ot")
                for c in range(CP):
                    def emit_scores_exp(i, c, j):
                        s_a = psp.tile([P, IB], f32, tag="s", name="s_a")
                        s_b = psp.tile([P, IB], f32, tag="s", name="s_b")
                        ksl = slice(j * P, (j + 1) * P)
                        for iq in range(IQ):
                            isl = slice(i * IB + iq * 512, i * IB + (iq + 1) * 512)
                            osl = slice(iq * 512, (iq + 1) * 512)
                            # row-packed score matmuls: head A rows 0:64,
                            # head B rows 64:128 of qk feature chunk c
                            nc.tensor.matmul(s_a[:, osl], kt[0:64, c, ksl],
                                             qt[0:64, c, isl], start=True, stop=True)
                            nc.tensor.matmul(s_b[:, osl], kt[64:128, c, ksl],
                                             qt[64:128, c, isl], start=True, stop=True)
                        p_a = ppool.tile([P, IB], bf16, tag="p", name="p_a")
                        nc.scalar.activation(p_a[:], s_a[:], AF.Exp, scale=SCALE)
                        p_b = ppool.tile([P, IB], bf16, tag="p", name="p_b")
                        nc.scalar.activation(p_b[:], s_b[:], AF.Exp, scale=SCALE)
                        return p_a, p_b

                    def emit_pv(oa, ob, p_a, p_b, j):
                        st = (j == 0)
                        sp = (j == TJ - 1)
                        # flipped PV: out O[i-chunk, d], lhsT = P~ slice,
                        # rhs = V||ones; column 64 accumulates the softmax
                        # denominator.  PSUM zeroing is per 2KB region (4
                        # ic-slots), so only the first ic of a region opens
                        # the accumulation group (start zeroes the whole
                        # region; later ics write through pending-zero) and
                        # only the last ic closes it.
                        for ic in range(IC):
                            nc.tensor.matmul(oa[:, ic, 0:65],
                                             p_a[:, ic * P:(ic + 1) * P],
                                             vsb[:, j, 2 * c, :],
                                             start=st and ic % 4 == 0,
                                             stop=sp and ic % 4 == 3)
                        for ic in range(IC):
                            nc.tensor.matmul(ob[:, ic, 0:65],
                                             p_b[:, ic * P:(ic + 1) * P],
                                             vsb[:, j, 2 * c + 1, :],
                                             start=st and ic % 4 == 0,
                                             stop=sp and ic % 4 == 3)

                    # 1-j head start + scores(j+1) emitted before PV(j):
                    # decouples the exp stream from the oa/ob slot-release
                    # chain of the previous unit
                    head0 = emit_scores_exp(i, c, 0)
                    for _ in range(2):
                        if fillers:
                            fillers.popleft()()
                    if i == 0 and c == CP - 1:
                        # all Q fillers have been emitted; release the
                        # weight-stream pool and load W_proj into its bytes
                        ws_stack.close()
                        p2 = c3.enter_context(tc.tile_pool(name="p2", bufs=1))
                        wp_sb = p2.tile([P, DH // P, D], bf16)
                        nc.sync.dma_start(
                            wp_sb[:], wp.rearrange("(c p) o -> p c o", p=P))
                    oa = psp.tile([P, IC, P], f32, tag="oa", bufs=1, name="oa")
                    ob = psp.tile([P, IC, P], f32, tag="ob", bufs=1, name="ob")
                    p_prev = head0
                    for j in range(1, TJ):
                        p_cur = emit_scores_exp(i, c, j)
                        emit_pv(oa, ob, p_prev[0], p_prev[1], j - 1)
                        p_prev = p_cur
                    emit_pv(oa, ob, p_prev[0], p_prev[1], TJ - 1)
                    # deferred softmax normalization: O[i, d] * (1/denom),
                    # denom is a per-partition scalar (column 64)
                    ra = dpool.tile([P, IC], f32, tag="ra")
                    nc.vector.reciprocal(ra[:], oa[:, :, 64])
                    rb = dpool.tile([P, IC], f32, tag="rb")
                    nc.vector.reciprocal(rb[:], ob[:, :, 64])
                    # normalize + transpose each [128 i, 128 d-pair] tile
                    # back to O^T; all 8 transposes pack into one s-slot
                    tp = psp.tile([P, IB], f32, tag="s", name="tp")
                    for ic in range(IC):
                        on = onpool.tile([P, P], bf16, tag="on")
                        nc.vector.tensor_scalar_mul(
                            on[:, 0:64], oa[:, ic, 0:64], ra[:, ic:ic + 1])
                        nc.vector.tensor_scalar_mul(
                            on[:, 64:128], ob[:, ic, 0:64], rb[:, ic:ic + 1])
                        # all 8 transposed blocks live in one 2KB psum
                        # region: open/close its accumulation group once
                        nc.tensor.matmul(
                            tp[:, 64 * ic:64 * (ic + 1)].bitcast(bf16),
                            on[:], ident[:], is_transpose=True,
                            start=(ic == 0), stop=(ic == IC - 1))
                    nc.vector.tensor_copy(ot_i[:, c, :],
                                          tp[:, 0:512].bitcast(bf16))
                emit_proj(i, ot_i)
            # drain remaining fillers (tail projection work)
            while fillers:
                fillers.popleft()()

    nc.compile()
    return nc


def _get_nc():
    if "nc" not in _cached:
        _cached["nc"] = _build()
    return _cached["nc"]


def kernel(x, W_qkv, b_qkv, W_proj, b_proj):
    from concourse.bass_utils import run_bass_kernel_spmd

    x = np.asarray(x, dtype=np.float32)
    W_qkv = np.asarray(W_qkv, dtype=np.float32)
    b_qkv = np.asarray(b_qkv, dtype=np.float32)
    W_proj = np.asarray(W_proj, dtype=np.float32)
    b_proj = np.asarray(b_proj, dtype=np.float32)

    in_maps = []
    for core in range(NCORES):
        b, hg = divmod(core, HG)
        hs = slice(DH * hg, DH * (hg + 1))
        in_maps.append({
            "xt": np.ascontiguousarray(x[b].T),
            "wqk": np.ascontiguousarray(
                np.concatenate([W_qkv[:, hs],
                                W_qkv[:, D + DH * hg:D + DH * (hg + 1)]], axis=1)),
            "wv": np.ascontiguousarray(W_qkv[:, 2 * D + DH * hg:2 * D + DH * (hg + 1)]),
            "wp": np.ascontiguousarray(
                W_proj[DH * hg:DH * (hg + 1), :].astype(ml_dtypes.bfloat16)),
            "bqk": np.concatenate([b_qkv[hs],
                                   b_qkv[D + DH * hg:D + DH * (hg + 1)]])[None, :],
            "bv": b_qkv[2 * D + DH * hg:2 * D + DH * (hg + 1)][None, :],
        })

    nc = _get_nc()
    res = run_bass_kernel_spmd(nc, in_maps, core_ids=list(range(NCORES)))
    out = np.empty((B, N, D), dtype=np.float32)
    for b in range(B):
        out[b] = res.results[2 * b]["y"] + res.results[2 * b + 1]["y"] + b_proj
    return out


# revision 15
# speedup vs baseline: 1.0312x; 1.0312x over previous
"""Multi-head attention (B=4, N=2048, D=1024, H=16) on 8 Trainium2 cores.

Sharding: core = (batch b, head-group hg) -> 4 batches x 2 groups of 8 heads.

Per-core pipeline (all activations/weights stream as bf16; PSUM stays f32):
  - K^T preamble: the only serial prefix (scores for j need all of K).
    x^T streams in n-chunks of 512 tokens, all four stay SBUF-resident.
  - Q^T chunk (c0, i-block 0) closes the preamble; attention starts ~30us in.
  - 8 attention units (i-block x head-pair), each 16 j-slots of
    scores (row-packed K=64 matmul pairs) + exp (scalar engine, no
    max-subtraction: scores ~N(0, 0.17) and softmax is shift-invariant).
  - PV in flipped orientation: O[i, d] with M=128 output partitions (PE cost
    is proportional to the moving free size only, so M=128 halves PE time vs
    the M=65 O^T form).  lhsT = P~ slice [j, i-chunk], rhs = V||ones, so
    column 64 of the accumulator is the softmax denominator for free.
    PSUM zeroing is per 2KB region: one start/stop per 4-ic region.
  - deferred normalization O * (1/denom) on the DVE (per-partition scalar),
    then PE transposes back to O^T (8 per unit packed into one PSUM region)
    feeding the output projection, which emits per-128-token partials.
  - V projection, remaining Q^T chunks, and the output projection are
    sliced into ~512-cycle micro-steps and emitted by a budget scheduler
    that keeps the PE stream just behind the ACT (exp) pace, in deadline
    order; PV emission lags exp by >= 2 slots and is forced beyond 6 to
    recycle the P~ ring.
  - V bias rides on the DVE PSUM->SBUF copy against a one-time
    gpsimd-broadcast bias tile (b_qkv is zero here, but stays honest).
Host sums the two per-batch partials and adds b_proj.
"""

import sys

if "/opt/trn_rl_repo" not in sys.path:
    sys.path.insert(0, "/opt/trn_rl_repo")

from collections import deque
from contextlib import ExitStack

import ml_dtypes
import numpy as np

B, N, D, H = 4, 2048, 1024, 16
HG = 2                 # head groups (tensor parallel)
NCORES = B * HG        # 8
DH = D // HG           # 512 features per group = 8 heads * 64
P = 128
KC = D // P            # 8 contraction chunks over d_model
NT = N // 512          # 4 token 512-chunks
TJ = N // P            # 16 token 128-chunks (the attention j axis)
IB = 1024              # i-block (exp free-dim)
NI = N // IB           # 2
IQ = IB // 512         # 2 matmul free-dim halves per i-block
IC = IB // P           # 8 i-chunks of 128 per i-block
CP = 4                 # head pairs per core
SCALE = (D // H) ** -0.5

# scheduler constants (PE cycles @2.4GHz)
EXP_SLOT = 4984        # ACT time per j-slot (2 exps of [128,1024] from PSUM)
DMA_LEAD = 6700        # first x chunk DMA latency before the preamble runs
LAG_MIN = 2            # PV lags exp by >= 2 slots (keeps scores ahead)
LAG_FORCE = 6          # force PV beyond this backlog (P~ ring is 8 slots)

_cached = {}


def _build():
    import concourse.mybir as mybir
    import concourse.tile as tile
    from concourse import bacc
    from concourse.masks import make_identity

    f32 = mybir.dt.float32
    bf16 = mybir.dt.bfloat16
    AF = mybir.ActivationFunctionType

    nc = bacc.Bacc("TRN2", target_bir_lowering=False, debug=False,
                   enable_asserts=False)

    xt = nc.dram_tensor("xt", (D, N), bf16, kind="ExternalInput").ap()
    wqk = nc.dram_tensor("wqk", (D, 2 * DH), bf16, kind="ExternalInput").ap()
    wv = nc.dram_tensor("wv", (D, DH), bf16, kind="ExternalInput").ap()
    wp = nc.dram_tensor("wp", (DH, D), bf16, kind="ExternalInput").ap()
    bqk = nc.dram_tensor("bqk", (1, 2 * DH), f32, kind="ExternalInput").ap()
    bv = nc.dram_tensor("bv", (1, DH), f32, kind="ExternalInput").ap()
    y = nc.dram_tensor("y", (N, D), f32, kind="ExternalOutput").ap()

    with tile.TileContext(nc) as tc, ExitStack() as ctx:
        const = ctx.enter_context(tc.tile_pool(name="const", bufs=1))
        persist = ctx.enter_context(tc.tile_pool(name="persist", bufs=1))
        ppool = ctx.enter_context(tc.tile_pool(name="pp", bufs=16))
        otpool = ctx.enter_context(tc.tile_pool(name="ot", bufs=2))
        dpool = ctx.enter_context(tc.tile_pool(name="dv", bufs=2))
        onpool = ctx.enter_context(tc.tile_pool(name="on", bufs=3))
        ypool = ctx.enter_context(tc.tile_pool(name="yb", bufs=3))
        xpool = ctx.enter_context(tc.tile_pool(name="xs", bufs=4))
        wpool = ctx.enter_context(tc.tile_pool(name="ws", bufs=1))
        # PSUM: tag "s" = 2x [128, IB] f32 ring (scores / qkv / proj /
        # transpose-out), oa/ob = per-head O[i, d] accumulators
        # ([128, 8, 128] f32 = 2 banks each) -> exactly 8 banks
        psp = ctx.enter_context(tc.tile_pool(name="psp", bufs=2, space="PSUM"))

        ones_f32 = const.tile([1, P], f32)
        nc.vector.memset(ones_f32[:], 1.0)
        bqk_sb = const.tile([P, 1, 2 * DH // P], f32)
        nc.sync.dma_start(bqk_sb[:], bqk.rearrange("a (mo p) -> p a mo", p=P))
        bv_sb = const.tile([1, DH], f32)
        nc.sync.dma_start(bv_sb[:], bv)
        bvb = const.tile([P, DH], f32)
        nc.gpsimd.partition_broadcast(bvb[:], bv_sb[:])
        bvb_r = bvb[:].rearrange("p (h d) -> p h d", d=64)
        ident = const.tile([P, P], bf16)
        make_identity(nc, ident[:])
        # preload the exp table
        dummy = const.tile([1, 16], f32)
        nc.scalar.activation(dummy[:], ones_f32[0:1, 0:16], AF.Exp)

        qt = persist.tile([P, CP, N], bf16)           # Q^T  [128, 4, 2048]
        kt = persist.tile([P, CP, N], bf16)           # K^T  [128, 4, 2048]
        # V with a ones column per head (65-wide head slots)
        vsb = persist.tile([P, TJ, H // HG, 65], bf16)
        nc.vector.memset(vsb[:, :, :, 64:65], 1.0)

        xt_r = xt.rearrange("(ko p) t -> p ko t", p=P)
        wqk_r = wqk.rearrange("(ko p) m -> p ko m", p=P)
        wv_r = wv.rearrange("(ko p) m -> p ko m", p=P)

        # ---- input DMAs: wk interleaved with x n3 (the first K chunk),
        # then the rest; everything stays resident ----
        wk_sb = wpool.tile([P, KC, DH], bf16, tag="wk")
        xts = {}
        xts[NT - 1] = xpool.tile([P, KC, 512], bf16, tag="xt", name="xt_n")
        for k in range(KC):
            nc.sync.dma_start(wk_sb[:, k, :], wqk_r[:, k, DH:2 * DH])
            nc.sync.dma_start(xts[NT - 1][:, k, :],
                              xt_r[:, k, (NT - 1) * 512:NT * 512])
        for n in (2, 1, 0):
            xts[n] = xpool.tile([P, KC, 512], bf16, tag="xt", name="xt_n")
            for k in range(KC):
                nc.sync.dma_start(xts[n][:, k, :],
                                  xt_r[:, k, n * 512:(n + 1) * 512])
        wq_sb = wpool.tile([P, KC, DH], bf16, tag="wq")
        for k in range(KC):
            nc.sync.dma_start(wq_sb[:, k, :], wqk_r[:, k, 0:DH])
        wv_sb = wpool.tile([P, KC, DH], bf16, tag="wv")
        for k in range(KC):
            nc.sync.dma_start(wv_sb[:, k, :], wv_r[:, k, :])
        wp_sb = wpool.tile([P, DH // P, D], bf16, tag="wp")
        nc.sync.dma_start(wp_sb[:], wp.rearrange("(c p) o -> p c o", p=P))

        # ---- scheduler state ----
        st = {"pe": 0, "act": None}

        def pe_add(cyc):
            st["pe"] += cyc

        # ---- emitters ----
        def emit_kq(dst, w_sb, c, n, bias_off):
            pt = psp.tile([P, IB], f32, tag="s", name="pt")
            for k in range(KC):
                nc.tensor.matmul(pt[:, 0:512], w_sb[:, k, c * P:(c + 1) * P],
                                 xts[n][:, k, :], start=(k == 0),
                                 stop=(k == KC - 1))
            pe_add(KC * 512)
            nc.vector.tensor_scalar_add(
                dst[:, c, n * 512:(n + 1) * 512], pt[:, 0:512],
                bqk_sb[:, 0, bias_off + c:bias_off + c + 1])

        def emit_v(j):
            n, tt = divmod(j, 4)
            pv = psp.tile([P, IB], f32, tag="s", name="pv")
            for k in range(KC):
                nc.tensor.matmul(pv[:, 0:DH],
                                 xts[n][:, k, tt * P:(tt + 1) * P],
                                 wv_sb[:, k, :], start=(k == 0),
                                 stop=(k == KC - 1))
            pe_add(KC * 512)
            nc.vector.tensor_add(
                vsb[:, j, :, 0:64],
                pv[:, 0:DH].rearrange("p (h d) -> p h d", d=64), bvb_r)

        # ---- K preamble + first Q chunk ----
        for n in range(NT - 1, -1, -1):
            for c in range(CP):
                emit_kq(kt, wk_sb, c, n, CP)
        emit_kq(qt, wq_sb, 0, 0, 0)
        emit_kq(qt, wq_sb, 0, 1, 0)

        # ---- filler queue: (deadline_unit, kind, emit_fn); EDF order.
        # kind "v" pops advance v_done (PV(unit0, j) needs V(j) emitted
        # first - PE executes in emission order) ----
        fillers = deque()
        for j in range(TJ):
            fillers.append((1, "v", lambda j=j: emit_v(j)))
        qsched = [(1, 0), (1, 1), (2, 0), (2, 1), (3, 0), (3, 1),
                  (0, 2), (0, 3), (1, 2), (1, 3),
                  (2, 2), (2, 3), (3, 2), (3, 3)]
        for idx, (c, n) in enumerate(qsched):
            dl = 1 + idx // 2
            fillers.append((dl, "q",
                            lambda c=c, n=n: emit_kq(qt, wq_sb, c, n, 0)))

        def emit_proj(i, ot_blk, t, o):
            yp_full = psp.tile([P, IB], f32, tag="s", name="yp")
            yp = yp_full[:, 0:512]
            for cc in range(CP):
                nc.tensor.matmul(yp[:], ot_blk[:, cc, t * P:(t + 1) * P],
                                 wp_sb[:, cc, o * 512:(o + 1) * 512],
                                 start=(cc == 0), stop=(cc == CP - 1))
            pe_add(CP * 512)
            ysb = ypool.tile([P, 512], f32, tag="y")
            nc.vector.tensor_copy(ysb[:], yp[:])
            r0 = i * IB + t * P
            nc.sync.dma_start(y[r0:r0 + P, o * 512:(o + 1) * 512], ysb[:])

        def pop_filler():
            _, kind, fn = fillers.popleft()
            fn()
            if kind == "v":
                prog["v_done"] += 1

        # ---- attention units ----
        units = [(i, c) for i in range(NI) for c in range(CP)]
        pvq = deque()          # (uidx, j, p_a, p_b) exp emitted, PV pending
        ustate = {}            # uidx -> dict(oa, ob, npv, i, c, ot)
        prog = {"v_done": 0, "norm_done": -1}
        ot_blks = {}

        def emit_scores_exp(i, c, j):
            s_a = psp.tile([P, IB], f32, tag="s", name="s_a")
            s_b = psp.tile([P, IB], f32, tag="s", name="s_b")
            ksl = slice(j * P, (j + 1) * P)
            for iq in range(IQ):
                isl = slice(i * IB + iq * 512, i * IB + (iq + 1) * 512)
                osl = slice(iq * 512, (iq + 1) * 512)
                nc.tensor.matmul(s_a[:, osl], kt[0:64, c, ksl],
                                 qt[0:64, c, isl], start=True, stop=True)
                nc.tensor.matmul(s_b[:, osl], kt[64:128, c, ksl],
                                 qt[64:128, c, isl], start=True, stop=True)
            pe_add(IQ * 1024)
            p_a = ppool.tile([P, IB], bf16, tag="p", name="p_a")
            nc.scalar.activation(p_a[:], s_a[:], AF.Exp, scale=SCALE)
            p_b = ppool.tile([P, IB], bf16, tag="p", name="p_b")
            nc.scalar.activation(p_b[:], s_b[:], AF.Exp, scale=SCALE)
            return p_a, p_b

        def pv_front_eligible():
            if not pvq:
                return False
            uidx, j, _, _ = pvq[0]
            if prog["norm_done"] < uidx - 1:
                return False
            return uidx > 0 or prog["v_done"] > j

        def emit_norm(uidx):
            us = ustate[uidx]
            i, c = us["i"], us["c"]
            oa, ob = us["oa"], us["ob"]
            ot_i = ot_blks[i]
            ra = dpool.tile([P, IC], f32, tag="ra")
            nc.vector.reciprocal(ra[:], oa[:, :, 64])
            rb = dpool.tile([P, IC], f32, tag="rb")
            nc.vector.reciprocal(rb[:], ob[:, :, 64])
            tp = psp.tile([P, IB], f32, tag="s", name="tp")
            for ic in range(IC):
                on = onpool.tile([P, P], bf16, tag="on")
                nc.vector.tensor_scalar_mul(
                    on[:, 0:64], oa[:, ic, 0:64], ra[:, ic:ic + 1])
                nc.vector.tensor_scalar_mul(
                    on[:, 64:128], ob[:, ic, 0:64], rb[:, ic:ic + 1])
                # all 8 transposed blocks share one 2KB psum region:
                # open/close its accumulation group once
                nc.tensor.matmul(
                    tp[:, 64 * ic:64 * (ic + 1)].bitcast(bf16),
                    on[:], ident[:], is_transpose=True,
                    start=(ic == 0), stop=(ic == IC - 1))
            pe_add(IC * P)
            nc.vector.tensor_copy(ot_i[:, c, :], tp[:, 0:512].bitcast(bf16))
            prog["norm_done"] = uidx
            if c == CP - 1:
                for t in range(IB // P):
                    for o in range(D // 512):
                        fillers.append(
                            (10 ** 9, "proj",
                             lambda i=i, ot=ot_i, t=t, o=o: emit_proj(i, ot, t, o)))

        def emit_pv_one():
            uidx, j, p_a, p_b = pvq.popleft()
            us = ustate[uidx]
            if us["oa"] is None:
                us["oa"] = psp.tile([P, IC, P], f32, tag="oa", bufs=1, name="oa")
                us["ob"] = psp.tile([P, IC, P], f32, tag="ob", bufs=1, name="ob")
            oa, ob = us["oa"], us["ob"]
            c = us["c"]
            stt = (j == 0)
            sp = (j == TJ - 1)
            # PSUM zeroing is per 2KB region (4 ic-slots): only the first ic
            # of a region opens the group, only the last closes it
            for ic in range(IC):
                nc.tensor.matmul(oa[:, ic, 0:65], p_a[:, ic * P:(ic + 1) * P],
                                 vsb[:, j, 2 * c, :],
                                 start=stt and ic % 4 == 0,
                                 stop=sp and ic % 4 == 3)
            for ic in range(IC):
                nc.tensor.matmul(ob[:, ic, 0:65], p_b[:, ic * P:(ic + 1) * P],
                                 vsb[:, j, 2 * c + 1, :],
                                 start=stt and ic % 4 == 0,
                                 stop=sp and ic % 4 == 3)
            pe_add(2 * IC * 65)
            us["npv"] += 1
            if us["npv"] == TJ:
                emit_norm(uidx)

        for uidx, (i, c) in enumerate(units):
            ustate[uidx] = {"i": i, "c": c, "oa": None, "ob": None, "npv": 0}
            if c == 0:
                ot_blks[i] = otpool.tile([P, CP, IB], bf16, tag="ot",
                                         name="ot_i")
            # deadline forcing: everything due by this unit must be in the
            # PE stream before its scores
            while fillers and fillers[0][0] <= uidx:
                pop_filler()
            for j in range(TJ):
                p_a, p_b = emit_scores_exp(i, c, j)
                pvq.append((uidx, j, p_a, p_b))
                if st["act"] is None:
                    st["act"] = st["pe"] + DMA_LEAD
                st["act"] += EXP_SLOT
                # P~ ring forcing
                while len(pvq) > LAG_FORCE:
                    if pv_front_eligible():
                        emit_pv_one()
                    elif fillers:
                        pop_filler()
                    else:
                        break
                # budget fillers: keep PE just behind the ACT stream
                while st["pe"] < st["act"]:
                    if pv_front_eligible() and len(pvq) > LAG_MIN:
                        emit_pv_one()
                    elif fillers:
                        pop_filler()
                    elif pv_front_eligible():
                        emit_pv_one()
                    else:
                        break
        # ---- tail: drain PVs then remaining fillers ----
        while pvq:
            if pv_front_eligible():
                emit_pv_one()
            elif fillers:
                pop_filler()
            else:
                raise RuntimeError("scheduler deadlock")
        while fillers:
            pop_filler()

    nc.compile()
    return nc


def _get_nc():
    if "nc" not in _cached:
        _cached["nc"] = _build()
    return _cached["nc"]


def kernel(x, W_qkv, b_qkv, W_proj, b_proj):
    from concourse.bass_utils import run_bass_kernel_spmd

    x = np.asarray(x, dtype=np.float32)
    W_qkv = np.asarray(W_qkv, dtype=np.float32)
    b_qkv = np.asarray(b_qkv, dtype=np.float32)
    W_proj = np.asarray(W_proj, dtype=np.float32)
    b_proj = np.asarray(b_proj, dtype=np.float32)
    bf = ml_dtypes.bfloat16

    in_maps = []
    for core in range(NCORES):
        b, hg = divmod(core, HG)
        hs = slice(DH * hg, DH * (hg + 1))
        in_maps.append({
            "xt": np.ascontiguousarray(x[b].T.astype(bf)),
            "wqk": np.ascontiguousarray(
                np.concatenate([W_qkv[:, hs],
                                W_qkv[:, D + DH * hg:D + DH * (hg + 1)]],
                               axis=1).astype(bf)),
            "wv": np.ascontiguousarray(
                W_qkv[:, 2 * D + DH * hg:2 * D + DH * (hg + 1)].astype(bf)),
            "wp": np.ascontiguousarray(
                W_proj[DH * hg:DH * (hg + 1), :].astype(bf)),
            "bqk": np.concatenate([b_qkv[hs],
                                   b_qkv[D + DH * hg:D + DH * (hg + 1)]])[None, :],
            "bv": b_qkv[2 * D + DH * hg:2 * D + DH * (hg + 1)][None, :],
        })

    nc = _get_nc()
    res = run_bass_kernel_spmd(nc, in_maps, core_ids=list(range(NCORES)))
    out = np.empty((B, N, D), dtype=np.float32)
    for b in range(B):
        out[b] = res.results[2 * b]["y"] + res.results[2 * b + 1]["y"] + b_proj
    return out


# revision 19
# speedup vs baseline: 1.0882x; 1.0552x over previous
"""Multi-head attention (B=4, N=2048, D=1024, H=16) on 8 Trainium2 cores.

Sharding: core = (batch b, head-group hg) -> 4 batches x 2 groups of 8 heads.

Per-core pipeline (all activations/weights stream as bf16; PSUM stays f32):
  - K^T preamble: the only serial prefix (scores for j need all of K).
    x^T streams in n-chunks of 512 tokens, all four stay SBUF-resident.
  - Q^T chunk (c0, i-block 0) closes the preamble; attention starts ~30us in.
  - 8 attention units (i-block x head-pair), each 16 j-slots of
    scores (row-packed K=64 matmul pairs) + exp (scalar engine, no
    max-subtraction: scores ~N(0, 0.17) and softmax is shift-invariant).
  - PV in flipped orientation: O[i, d] with M=128 output partitions (PE cost
    is proportional to the moving free size only, so M=128 halves PE time vs
    the M=65 O^T form).  lhsT = P~ slice [j, i-chunk], rhs = V||ones, so
    column 64 of the accumulator is the softmax denominator for free.
    PSUM zeroing is per 2KB region: one start/stop per 4-ic region.
  - deferred normalization O * (1/denom) on the DVE (per-partition scalar),
    then PE transposes back to O^T (8 per unit packed into one PSUM region)
    feeding the output projection, which emits per-128-token partials.
  - V projection, remaining Q^T chunks, and the output projection are
    sliced into ~512-cycle micro-steps and emitted by a budget scheduler
    that keeps the PE stream just behind the ACT (exp) pace, in deadline
    order; PV emission lags exp by >= 2 slots and is forced beyond 6 to
    recycle the P~ ring.
  - V bias rides on the DVE PSUM->SBUF copy against a one-time
    gpsimd-broadcast bias tile (b_qkv is zero here, but stays honest).
Host sums the two per-batch partials and adds b_proj.
"""

import sys

if "/opt/trn_rl_repo" not in sys.path:
    sys.path.insert(0, "/opt/trn_rl_repo")

from collections import deque
from contextlib import ExitStack

import ml_dtypes
import numpy as np

B, N, D, H = 4, 2048, 1024, 16
HG = 2                 # head groups (tensor parallel)
NCORES = B * HG        # 8
DH = D // HG           # 512 features per group = 8 heads * 64
P = 128
KC = D // P            # 8 contraction chunks over d_model
NT = N // 512          # 4 token 512-chunks
TJ = N // P            # 16 token 128-chunks (the attention j axis)
IB = 1024              # i-block (exp free-dim)
NI = N // IB           # 2
IQ = IB // 512         # 2 matmul free-dim halves per i-block
IC = IB // P           # 8 i-chunks of 128 per i-block
CP = 4                 # head pairs per core
SCALE = (D // H) ** -0.5

# scheduler constants (PE cycles @2.4GHz)
EXP_SLOT = 2492        # ACT time per j-slot (1 exp of [128,1024] from PSUM)
DMA_LEAD = 6700        # first x chunk DMA latency before the preamble runs
LAG_MIN = 2            # PV lags exp by >= 2 slots (keeps scores ahead)
LAG_FORCE = 12         # force PV beyond this backlog (P~ ring is 16 slots)

_cached = {}


def _build():
    import concourse.mybir as mybir
    import concourse.tile as tile
    from concourse import bacc
    from concourse.masks import make_identity

    f32 = mybir.dt.float32
    bf16 = mybir.dt.bfloat16
    AF = mybir.ActivationFunctionType

    nc = bacc.Bacc("TRN2", target_bir_lowering=False, debug=False,
                   enable_asserts=False)

    xt = nc.dram_tensor("xt", (D, N), bf16, kind="ExternalInput").ap()
    wqk = nc.dram_tensor("wqk", (D, 2 * DH), bf16, kind="ExternalInput").ap()
    wv = nc.dram_tensor("wv", (D, DH), bf16, kind="ExternalInput").ap()
    wp = nc.dram_tensor("wp", (DH, D), bf16, kind="ExternalInput").ap()
    bqk = nc.dram_tensor("bqk", (1, 2 * DH), f32, kind="ExternalInput").ap()
    bv = nc.dram_tensor("bv", (1, DH), f32, kind="ExternalInput").ap()
    y = nc.dram_tensor("y", (N, D), f32, kind="ExternalOutput").ap()

    with tile.TileContext(nc) as tc, ExitStack() as ctx:
        const = ctx.enter_context(tc.tile_pool(name="const", bufs=1))
        persist = ctx.enter_context(tc.tile_pool(name="persist", bufs=1))
        ppool = ctx.enter_context(tc.tile_pool(name="pp", bufs=16))
        otpool = ctx.enter_context(tc.tile_pool(name="ot", bufs=2))
        dpool = ctx.enter_context(tc.tile_pool(name="dv", bufs=2))
        onpool = ctx.enter_context(tc.tile_pool(name="on", bufs=3))
        ypool = ctx.enter_context(tc.tile_pool(name="yb", bufs=3))
        xpool = ctx.enter_context(tc.tile_pool(name="xs", bufs=4))
        wpool = ctx.enter_context(tc.tile_pool(name="ws", bufs=1))
        # PSUM: tag "s" = 2x [128, IB] f32 ring (scores / transpose-out),
        # tag "oa" = single-head O[i, d] accumulator ([128, 8, 128] f32 =
        # 2 banks), tag "f" = 2x [128, 512] f32 filler ring (qkv / proj)
        # -> exactly 8 banks
        psp = ctx.enter_context(tc.tile_pool(name="psp", bufs=2, space="PSUM"))

        ones_f32 = const.tile([1, P], f32)
        nc.vector.memset(ones_f32[:], 1.0)
        bqk_sb = const.tile([P, 1, 2 * DH // P], f32)
        nc.sync.dma_start(bqk_sb[:], bqk.rearrange("a (mo p) -> p a mo", p=P))
        bv_sb = const.tile([1, DH], f32)
        nc.sync.dma_start(bv_sb[:], bv)
        bvb = const.tile([P, DH], f32)
        nc.gpsimd.partition_broadcast(bvb[:], bv_sb[:])
        bvb_r = bvb[:].rearrange("p (h d) -> p h d", d=64)
        ident = const.tile([P, P], bf16)
        make_identity(nc, ident[:])
        # preload the exp table
        dummy = const.tile([1, 16], f32)
        nc.scalar.activation(dummy[:], ones_f32[0:1, 0:16], AF.Exp)

        qt = persist.tile([P, CP, N], bf16)           # Q^T  [128, 4, 2048]
        kt = persist.tile([P, CP, N], bf16)           # K^T  [128, 4, 2048]
        # V with a ones column per head (65-wide head slots)
        vsb = persist.tile([P, TJ, H // HG, 65], bf16)
        nc.vector.memset(vsb[:, :, :, 64:65], 1.0)

        xt_r = xt.rearrange("(ko p) t -> p ko t", p=P)
        wqk_r = wqk.rearrange("(ko p) m -> p ko m", p=P)
        wv_r = wv.rearrange("(ko p) m -> p ko m", p=P)

        # ---- input DMAs: wk interleaved with x n3 (the first K chunk),
        # then the rest; everything stays resident ----
        wk_sb = wpool.tile([P, KC, DH], bf16, tag="wk")
        xts = {}
        xts[NT - 1] = xpool.tile([P, KC, 512], bf16, tag="xt", name="xt_n")
        for k in range(KC):
            nc.sync.dma_start(wk_sb[:, k, :], wqk_r[:, k, DH:2 * DH])
            nc.sync.dma_start(xts[NT - 1][:, k, :],
                              xt_r[:, k, (NT - 1) * 512:NT * 512])
        for n in (2, 1, 0):
            xts[n] = xpool.tile([P, KC, 512], bf16, tag="xt", name="xt_n")
            for k in range(KC):
                nc.sync.dma_start(xts[n][:, k, :],
                                  xt_r[:, k, n * 512:(n + 1) * 512])
        wq_sb = wpool.tile([P, KC, DH], bf16, tag="wq")
        for k in range(KC):
            nc.sync.dma_start(wq_sb[:, k, :], wqk_r[:, k, 0:DH])
        wv_sb = wpool.tile([P, KC, DH], bf16, tag="wv")
        for k in range(KC):
            nc.sync.dma_start(wv_sb[:, k, :], wv_r[:, k, :])
        wp_sb = wpool.tile([P, DH // P, D], bf16, tag="wp")
        nc.sync.dma_start(wp_sb[:], wp.rearrange("(c p) o -> p c o", p=P))

        # ---- scheduler state ----
        st = {"pe": 0, "act": None}

        def pe_add(cyc):
            st["pe"] += cyc

        # ---- emitters ----
        def emit_kq(dst, w_sb, c, n, bias_off):
            pt = psp.tile([P, 512], f32, tag="f", name="pt")
            for k in range(KC):
                nc.tensor.matmul(pt[:], w_sb[:, k, c * P:(c + 1) * P],
                                 xts[n][:, k, :], start=(k == 0),
                                 stop=(k == KC - 1))
            pe_add(KC * 512)
            nc.vector.tensor_scalar_add(
                dst[:, c, n * 512:(n + 1) * 512], pt[:],
                bqk_sb[:, 0, bias_off + c:bias_off + c + 1])

        def emit_v(j):
            n, tt = divmod(j, 4)
            pv = psp.tile([P, 512], f32, tag="f", name="pv")
            for k in range(KC):
                nc.tensor.matmul(pv[:],
                                 xts[n][:, k, tt * P:(tt + 1) * P],
                                 wv_sb[:, k, :], start=(k == 0),
                                 stop=(k == KC - 1))
            pe_add(KC * 512)
            nc.vector.tensor_add(
                vsb[:, j, :, 0:64],
                pv[:].rearrange("p (h d) -> p h d", d=64), bvb_r)

        # ---- K preamble + first Q chunk ----
        for n in range(NT - 1, -1, -1):
            for c in range(CP):
                emit_kq(kt, wk_sb, c, n, CP)
        emit_kq(qt, wq_sb, 0, 0, 0)
        emit_kq(qt, wq_sb, 0, 1, 0)

        # ---- filler queue: (deadline_unit, kind, emit_fn); EDF order.
        # kind "v" pops advance v_done (PV(unit, j) needs V(j) emitted
        # first - PE executes in emission order) ----
        fillers = deque()
        for j in range(TJ):
            fillers.append((1, "v", lambda j=j: emit_v(j)))
        # Q chunk (c, n) is due at unit (i_of_n, h=2c); units are per head
        qsched = [(1, 0), (1, 1), (2, 0), (2, 1), (3, 0), (3, 1),
                  (0, 2), (0, 3), (1, 2), (1, 3),
                  (2, 2), (2, 3), (3, 2), (3, 3)]
        for c, n in qsched:
            dl = (n // 2) * 8 + 2 * c
            fillers.append((dl, "q",
                            lambda c=c, n=n: emit_kq(qt, wq_sb, c, n, 0)))
        fillers = deque(sorted(fillers, key=lambda f: f[0]))

        def emit_proj(i, ot_blk, t, o):
            yp = psp.tile([P, 512], f32, tag="f", name="yp")
            for cc in range(CP):
                nc.tensor.matmul(yp[:], ot_blk[:, cc, t * P:(t + 1) * P],
                                 wp_sb[:, cc, o * 512:(o + 1) * 512],
                                 start=(cc == 0), stop=(cc == CP - 1))
            pe_add(CP * 512)
            ysb = ypool.tile([P, 512], f32, tag="y")
            nc.vector.tensor_copy(ysb[:], yp[:])
            r0 = i * IB + t * P
            nc.sync.dma_start(y[r0:r0 + P, o * 512:(o + 1) * 512], ysb[:])

        def pop_filler():
            _, kind, fn = fillers.popleft()
            fn()
            if kind == "v":
                prog["v_done"] += 1

        # ---- attention units: one per (i-block, head) ----
        units = [(i, h) for i in range(NI) for h in range(H // HG)]
        pvq = deque()          # (uidx, j, p) exp emitted, PV pending
        ustate = {}            # uidx -> dict(oa, npv, i, h)
        prog = {"v_done": 0, "norm_done": -1}
        ot_blks = {}

        def emit_scores_exp(i, h, j):
            c, hb = divmod(h, 2)
            s = psp.tile([P, IB], f32, tag="s", name="s")
            ksl = slice(j * P, (j + 1) * P)
            for iq in range(IQ):
                isl = slice(i * IB + iq * 512, i * IB + (iq + 1) * 512)
                osl = slice(iq * 512, (iq + 1) * 512)
                nc.tensor.matmul(s[:, osl], kt[64 * hb:64 * hb + 64, c, ksl],
                                 qt[64 * hb:64 * hb + 64, c, isl],
                                 start=True, stop=True)
            pe_add(IQ * 512)
            p = ppool.tile([P, IB], bf16, tag="p", name="p")
            nc.scalar.activation(p[:], s[:], AF.Exp, scale=SCALE)
            return p

        def pv_front_eligible():
            if not pvq:
                return False
            uidx, j, _ = pvq[0]
            if prog["norm_done"] < uidx - 1:
                return False
            return uidx > 0 or prog["v_done"] > j

        def emit_norm(uidx):
            us = ustate[uidx]
            i, h = us["i"], us["h"]
            c, hb = divmod(h, 2)
            oa = us["oa"]
            ot_i = ot_blks[i]
            ra = dpool.tile([P, IC], f32, tag="ra")
            nc.vector.reciprocal(ra[:], oa[:, :, 64])
            tp = psp.tile([P, IB], f32, tag="s", name="tp")
            for ic in range(IC):
                on = onpool.tile([P, 64], bf16, tag="on")
                nc.vector.tensor_scalar_mul(
                    on[:], oa[:, ic, 0:64], ra[:, ic:ic + 1])
                # the 8 transposed [64, 128] blocks share one 2KB psum
                # region: open/close its accumulation group once
                nc.tensor.matmul(
                    tp[0:64, 64 * ic:64 * (ic + 1)].bitcast(bf16),
                    on[:], ident[:], is_transpose=True,
                    start=(ic == 0), stop=(ic == IC - 1))
            pe_add(IC * P)
            nc.vector.tensor_copy(ot_i[64 * hb:64 * hb + 64, c, :],
                                  tp[0:64, 0:512].bitcast(bf16))
            prog["norm_done"] = uidx
            if h == H // HG - 1:
                for t in range(IB // P):
                    for o in range(D // 512):
                        fillers.append(
                            (10 ** 9, "proj",
                             lambda i=i, ot=ot_i, t=t, o=o: emit_proj(i, ot, t, o)))

        def emit_pv_one():
            uidx, j, p = pvq.popleft()
            us = ustate[uidx]
            if us["oa"] is None:
                us["oa"] = psp.tile([P, IC, P], f32, tag="oa", bufs=1,
                                    name="oa")
            oa = us["oa"]
            h = us["h"]
            stt = (j == 0)
            sp = (j == TJ - 1)
            # PSUM zeroing is per 2KB region (4 ic-slots): only the first ic
            # of a region opens the group, only the last closes it
            for ic in range(IC):
                nc.tensor.matmul(oa[:, ic, 0:65], p[:, ic * P:(ic + 1) * P],
                                 vsb[:, j, h, :],
                                 start=stt and ic % 4 == 0,
                                 stop=sp and ic % 4 == 3)
            pe_add(IC * 65)
            us["npv"] += 1
            if us["npv"] == TJ:
                emit_norm(uidx)

        for uidx, (i, h) in enumerate(units):
            ustate[uidx] = {"i": i, "h": h, "oa": None, "npv": 0}
            if h == 0:
                ot_blks[i] = otpool.tile([P, CP, IB], bf16, tag="ot",
                                         name="ot_i")
            # deadline forcing: everything due by this unit must be in the
            # PE stream before its scores
            while fillers and fillers[0][0] <= uidx:
                pop_filler()
            for j in range(TJ):
                p = emit_scores_exp(i, h, j)
                pvq.append((uidx, j, p))
                if st["act"] is None:
                    st["act"] = st["pe"] + DMA_LEAD
                st["act"] += EXP_SLOT
                # P~ ring forcing
                while len(pvq) > LAG_FORCE:
                    if pv_front_eligible():
                        emit_pv_one()
                    elif fillers:
                        pop_filler()
                    else:
                        break
                # budget fillers: keep PE just behind the ACT stream
                while st["pe"] < st["act"]:
                    if pv_front_eligible() and len(pvq) > LAG_MIN:
                        emit_pv_one()
                    elif fillers:
                        pop_filler()
                    elif pv_front_eligible():
                        emit_pv_one()
                    else:
                        break
        # ---- tail: drain PVs then remaining fillers ----
        while pvq:
            if pv_front_eligible():
                emit_pv_one()
            elif fillers:
                pop_filler()
            else:
                raise RuntimeError("scheduler deadlock")
        while fillers:
            pop_filler()

    nc.compile()
    return nc


def _get_nc():
    if "nc" not in _cached:
        _cached["nc"] = _build()
    return _cached["nc"]


def kernel(x, W_qkv, b_qkv, W_proj, b_proj):
    from concourse.bass_utils import run_bass_kernel_spmd

    x = np.asarray(x, dtype=np.float32)
    W_qkv = np.asarray(W_qkv, dtype=np.float32)
    b_qkv = np.asarray(b_qkv, dtype=np.float32)
    W_proj = np.asarray(W_proj, dtype=np.float32)
    b_proj = np.asarray(b_proj, dtype=np.float32)
    bf = ml_dtypes.bfloat16

    in_maps = []
    for core in range(NCORES):
        b, hg = divmod(core, HG)
        hs = slice(DH * hg, DH * (hg + 1))
        in_maps.append({
            "xt": np.ascontiguousarray(x[b].T.astype(bf)),
            "wqk": np.ascontiguousarray(
                np.concatenate([W_qkv[:, hs],
                                W_qkv[:, D + DH * hg:D + DH * (hg + 1)]],
                               axis=1).astype(bf)),
            "wv": np.ascontiguousarray(
                W_qkv[:, 2 * D + DH * hg:2 * D + DH * (hg + 1)].astype(bf)),
            "wp": np.ascontiguousarray(
                W_proj[DH * hg:DH * (hg + 1), :].astype(bf)),
            "bqk": np.concatenate([b_qkv[hs],
                                   b_qkv[D + DH * hg:D + DH * (hg + 1)]])[None, :],
            "bv": b_qkv[2 * D + DH * hg:2 * D + DH * (hg + 1)][None, :],
        })

    nc = _get_nc()
    res = run_bass_kernel_spmd(nc, in_maps, core_ids=list(range(NCORES)))
    out = np.empty((B, N, D), dtype=np.float32)
    for b in range(B):
        out[b] = res.results[2 * b]["y"] + res.results[2 * b + 1]["y"] + b_proj
    return out
